# revision 13
# baseline (speedup 1.0000x reference)
"""Bidirectional cross-attention Trainium2 kernel.

Data-parallel over batch B=8 across 8 NeuronCores (1 sample/core).

Per-core dataflow (matmul operands fp16, attention weights bf16, fp32 accum):
  Softmax is invariant to per-row logit shifts, so the K projections fold
  into the Q side: S1 ~ Qt1^T x2 with Qt1 = (Wk2^T Wq1) x1 + Wk2^T bq1
  (host precomputes the folded [C,C] matrix + bias; x2 is used raw as the
  scores' stationary operand). The V bias folds into the residual:
  sum_j w_ij (V0_j + bv) = sum_j w_ij V0_j + bv, so the host adds bv to the
  transposed residual input and the V projection is bias-free; the
  denominator comes from a constant ones-column in the V^T tile.
  expS1T[j,i] = exp(x2^T Qt1)  (ScalarE exp -> bf16; no max-subtract: logits
                               bounded ~|33| for this problem's scale)
  outT[i, 0:257] = sum_j expS1T[j,i] * [V0T | 1][j, :]
     -> col 256 is the softmax denominator; y = outT[:,0:256]/denominator
        + (x1T + bv2)
  (symmetric for direction 2)
Chunks of 512 i-columns are software-pipelined: expS(k) matmuls+exps woven
with out(k-1) matmuls so PE never waits on ScalarE; dummy warm-up matmuls
keep the PE HAM clock ramping during the input DMA window.

Host side: shard batch over cores, fold weights, transpose x, gather +
transpose outputs.
"""

import sys

if "/opt/trn_rl_repo" not in sys.path:
    sys.path.insert(0, "/opt/trn_rl_repo")

import numpy as np

B, C, H, W = 8, 256, 48, 48
N = H * W  # 2304
NT = N // 128  # 18 j/i tiles
CT = C // 128  # 2 c tiles
CW = 512  # max i-chunk width for expS (last chunk is 256)
CHUNKS = [(0, 512), (512, 512), (1024, 512), (1536, 512), (2048, 256)]
N_WARM = 4

_CACHE = {}


def _build():
    import concourse.bacc as bacc
    import concourse.mybir as mybir
    from concourse.tile import TileContext

    F32, F16, BF16 = mybir.dt.float32, mybir.dt.float16, mybir.dt.bfloat16
    Exp = mybir.ActivationFunctionType.Exp
    Ident = mybir.ActivationFunctionType.Identity

    nc = bacc.Bacc(None, target_bir_lowering=False)

    x_d = {
        "x1": nc.dram_tensor("x1", [C, N], F16, kind="ExternalInput"),
        "x2": nc.dram_tensor("x2", [C, N], F16, kind="ExternalInput"),
    }
    xt_d = {
        "x1t": nc.dram_tensor("x1t", [N, C], F32, kind="ExternalInput"),
        "x2t": nc.dram_tensor("x2t", [N, C], F32, kind="ExternalInput"),
    }
    w_names = ["a1t", "wv2t", "a2t", "wv1t"]  # pack order
    wpack_d = nc.dram_tensor("wpack", [C, 4 * C], F16, kind="ExternalInput")
    bq_names = ["bq1", "bq2"]  # pack order (folded: v = Wk^T bq)
    # host-packed [128, ck, dir] so it loads in one DMA
    bq_d = nc.dram_tensor("bq", [128, CT * 2], F32, kind="ExternalInput")
    y_d = {
        "y1t": nc.dram_tensor("y1t", [N, C], F32, kind="ExternalOutput"),
        "y2t": nc.dram_tensor("y2t", [N, C], F32, kind="ExternalOutput"),
    }

    with TileContext(nc) as tc:
        with (
            tc.tile_pool(name="const", bufs=1) as cp,
            tc.tile_pool(name="proj", bufs=1) as pp,
            tc.tile_pool(name="stream", bufs=4) as sp,
            tc.tile_pool(name="psum", bufs=2, space="PSUM") as psp,
            tc.tile_pool(name="psum_s", bufs=3, space="PSUM") as psp2,
        ):
            # ---------- setup: warm-up + input loads ----------
            proj = {}
            # PE warm-up while input DMAs are in flight: ramps the HAM clock
            dummy = cp.tile([128, 512], F16, tag="warm")
            nc.vector.memset(dummy[:, :], 0.0)
            wps = None
            for _ in range(N_WARM):
                wps = psp.tile([128, 512], F32, tag="ps_o")
                nc.tensor.matmul(
                    wps[:, :], dummy[:, 0:128], dummy[:, :], start=True, stop=True
                )

            x_sb = {}

            def load_x(n, eng):
                # ck-interleaved halves: the first two DMAs cover both
                # c-tiles' low halves, unblocking the first proj chunks
                t = pp.tile([128, CT, N], F16, tag=n)
                for h0 in (0, N // 2):
                    for ck in range(CT):
                        eng.dma_start(
                            out=t[:, ck, h0 : h0 + N // 2],
                            in_=x_d[n][ck * 128 : (ck + 1) * 128, h0 : h0 + N // 2],
                        )
                x_sb[n] = t

            # input loads split across the two HWDGE queues: the Activation
            # queue (free until the first exp) carries everything Qt1 needs,
            # Sync carries x2 (needed a bit later by exp(c0))
            wpack = cp.tile([128, CT, 4 * C], F16, tag="wpack")
            for ck in range(CT):
                nc.scalar.dma_start(
                    out=wpack[:, ck, :], in_=wpack_d[ck * 128 : (ck + 1) * 128, :]
                )
            w_sb = {n: wpack[:, :, i * C : (i + 1) * C] for i, n in enumerate(w_names)}
            bqt = cp.tile([128, CT, 2], F32, tag="bqt")
            nc.scalar.dma_start(out=bqt[:, :, :], in_=bq_d[:, :])
            bq_sb = {n: bqt[:, :, i] for i, n in enumerate(bq_names)}
            load_x("x1", nc.scalar)
            load_x("x2", nc.sync)
            # preload the Exp activation table during the DMA window (emitted
            # after the scalar-queue DMA issues so it doesn't block them)
            wexp = cp.tile([128, 512], F32, tag="warm_exp")
            nc.scalar.activation(wexp[:, :], wps[:, :], Exp)

            # ---------- projection action builders ----------
            def proj_qk_actions(dst, xt, wn, bn, alt0=0):
                # chunk-major: the first two actions need only the low-half
                # x DMAs, so attention can start as early as possible
                acts = []
                i = 0
                for c0, cw in CHUNKS:
                    for ct in range(CT):

                        def mk(ct, c0, cw, use_act):
                            def act():
                                ps2 = psp2.tile([128, 2, CW], F32, tag="ps_s")
                                ps = ps2[:, 0, :]
                                for ck in range(CT):
                                    nc.tensor.matmul(
                                        ps[:, 0:cw],
                                        w_sb[wn][:, ck, ct * 128 : (ct + 1) * 128],
                                        xt[:, ck, c0 : c0 + cw],
                                        start=(ck == 0),
                                        stop=(ck == CT - 1),
                                    )
                                if use_act:
                                    nc.scalar.activation(
                                        dst[:, ct, c0 : c0 + cw],
                                        ps[:, 0:cw],
                                        Ident,
                                        bias=bq_sb[bn][:, ct : ct + 1],
                                    )
                                else:
                                    nc.vector.tensor_scalar_add(
                                        dst[:, ct, c0 : c0 + cw],
                                        ps[:, 0:cw],
                                        bq_sb[bn][:, ct : ct + 1],
                                    )

                            return act

                        acts.append(mk(ct, c0, cw, (alt0 + i) % 2 == 0))
                        i += 1
                return acts

            def proj_vt_actions(dst, xt, wn):
                # V^T tile: cols 0:256 = (W_v x)^T, col 256 = ones (memset at
                # setup; the V bias is folded into the residual on host)
                acts = []
                for jt in range(NT):

                    def mk(jt):
                        def act():
                            ps2 = psp2.tile([128, 2, CW], F32, tag="ps_s")
                            ps = ps2[:, 0, :]
                            for ck in range(CT):
                                nc.tensor.matmul(
                                    ps[:, 0:C],
                                    xt[:, ck, jt * 128 : (jt + 1) * 128],
                                    w_sb[wn][:, ck, :],
                                    start=(ck == 0),
                                    stop=(ck == CT - 1),
                                )
                            if jt % 3 != 0:
                                nc.vector.tensor_copy(dst[:, jt, 0:C], ps[:, 0:C])
                            else:
                                nc.scalar.activation(dst[:, jt, 0:C], ps[:, 0:C], Ident)

                        return act

                    acts.append(mk(jt))
                return acts

            for nm in ["Qt1", "Qt2"]:
                proj[nm] = pp.tile([128, CT, N], F16, tag=nm, name=nm)
            for nm in ["VT2", "VT1"]:
                proj[nm] = pp.tile([128, NT, C + 1], BF16, tag=nm, name=nm)
                nc.vector.memset(proj[nm][:, :, C : C + 1], 1.0)

            # only Qt1's first chunk must precede dir-1 attention (exp(c0)
            # reads Qt1[:, :, 0:512] only; the scores' stationary operand is
            # raw x2); the rest of Qt1, VT2 and all dir-2 projections become
            # fill work woven into dir-1's attention chunks. Qt1's chunk k+1
            # and all of VT2 must land within chunk k=0's weave (consumed by
            # exp(c1) / out(c0) from chunk 1 on) — quota0 covers both.
            qt1_acts = proj_qk_actions(proj["Qt1"], x_sb["x1"], "a1t", "bq1", 0)
            for a in qt1_acts[:2]:
                a()
            vt2_acts = proj_vt_actions(proj["VT2"], x_sb["x2"], "wv2t")
            fill = (
                qt1_acts[2:]
                + vt2_acts
                + proj_qk_actions(proj["Qt2"], x_sb["x2"], "a2t", "bq2", 1)
                + proj_vt_actions(proj["VT1"], x_sb["x1"], "wv1t")
            )
            n0 = len(qt1_acts) - 2 + len(vt2_acts)
            rest = len(fill) - n0
            quotas = [n0] + [(rest + 3) // 4] * 4

            # ---------- attention ----------
            with tc.tile_pool(name="ep", bufs=2) as ep:

                def exp_actions(Q, K, e, c0, cw):
                    # one action = expS matmuls + one wide exp for a PAIR of j-tiles
                    def mk(jp):
                        def act():
                            ps2 = psp2.tile([128, 2, CW], F32, tag="ps_s")
                            for jj in range(2):
                                jt = jp + jj
                                for ck in range(CT):
                                    nc.tensor.matmul(
                                        ps2[:, jj, 0:cw],
                                        K[:, ck, jt * 128 : (jt + 1) * 128],
                                        Q[:, ck, c0 : c0 + cw],
                                        start=(ck == 0),
                                        stop=(ck == CT - 1),
                                    )
                            nc.scalar.activation(
                                e[:, jp : jp + 2, 0:cw], ps2[:, :, 0:cw], Exp
                            )

                        return act

                    return [mk(jp) for jp in range(0, NT, 2)]

                def out_actions(e, VT, xt_dram, yt_dram, c0, cw):
                    # actions = out-matmul slices + epilogue, per i-subtile
                    acts = []
                    for il in range(cw // 128):
                        it = c0 // 128 + il
                        po = psp.tile([128, C + 1], F32, tag="ps_o")

                        xt_t = sp.tile([128, C], F32, tag="xt")

                        def mk_mm(po, il, it, j0, jn, xt_t):
                            def act():
                                if j0 == 0:
                                    nc.sync.dma_start(
                                        out=xt_t[:, :],
                                        in_=xt_dram[it * 128 : (it + 1) * 128, :],
                                    )
                                for jt in range(j0, jn):
                                    nc.tensor.matmul(
                                        po[:, :],
                                        e[:, jt, il * 128 : (il + 1) * 128],
                                        VT[:, jt, :],
                                        start=(jt == 0),
                                        stop=(jt == NT - 1),
                                    )

                            return act

                        for j0 in range(0, NT, 5):
                            acts.append(mk_mm(po, il, it, j0, min(j0 + 5, NT), xt_t))

                        def mk_epi(po, it, xt_t):
                            def act():
                                r = sp.tile([128, 1], F32, tag="r")
                                nc.vector.reciprocal(r[:, :], po[:, C : C + 1])
                                y = sp.tile([128, C], F32, tag="y")
                                nc.vector.scalar_tensor_tensor(
                                    y[:, :],
                                    po[:, 0:C],
                                    r[:, :],
                                    xt_t[:, :],
                                    op0=mybir.AluOpType.mult,
                                    op1=mybir.AluOpType.add,
                                )
                                nc.sync.dma_start(
                                    out=yt_dram[it * 128 : (it + 1) * 128, :], in_=y[:, :]
                                )

                            return act

                        acts.append(mk_epi(po, it, xt_t))
                    return acts

                def weave(a, b):
                    # emit all of a and b interleaved evenly (a paces, b fills)
                    if not b:
                        for f in a:
                            f()
                        return
                    na, nb = len(a), len(b)
                    j = 0
                    for i, f in enumerate(a):
                        f()
                        while j < nb and j * na <= (i + 1) * nb - 1:
                            b[j]()
                            j += 1
                    while j < nb:
                        b[j]()
                        j += 1

                # software pipeline: expS(k) woven with out(k-1); dir-2 projections
                # are distributed as extra fill across dir-1's chunks (they MUST
                # all be emitted before dir-2's first expS reads Qt2/VT1)
                plan = [
                    (proj["Qt1"], x_sb["x2"], proj["VT2"], xt_d["x1t"], y_d["y1t"], c0, cw)
                    for c0, cw in CHUNKS
                ] + [
                    (proj["Qt2"], x_sb["x1"], proj["VT1"], xt_d["x2t"], y_d["y2t"], c0, cw)
                    for c0, cw in CHUNKS
                ]
                nd1 = len(CHUNKS)
                pending = []
                for step, (Q, K, VT, xtd, ytd, c0, cw) in enumerate(plan):
                    if step < nd1:
                        q = quotas[step]
                        extra, fill = fill[:q], fill[q:]
                    else:
                        assert not fill
                        extra = []
                    e = ep.tile([128, NT, CW], BF16, tag="e")
                    weave(exp_actions(Q, K, e, c0, cw), pending + extra)
                    pending = out_actions(e, VT, xtd, ytd, c0, cw)
                weave(pending, [])

    nc.compile()
    return nc


def _get_nc():
    if "nc" not in _CACHE:
        _CACHE["nc"] = _build()
    return _CACHE["nc"]


def kernel(
    x1,
    x2,
    w_q1,
    b_q1,
    w_k1,
    b_k1,
    w_v1,
    b_v1,
    w_q2,
    b_q2,
    w_k2,
    b_k2,
    w_v2,
    b_v2,
    _trace=False,
):
    from concourse.bass_utils import run_bass_kernel_spmd

    nc = _get_nc()

    x1 = np.asarray(x1, dtype=np.float32)
    x2 = np.asarray(x2, dtype=np.float32)
    x1h = x1.astype(np.float16)
    x2h = x2.astype(np.float16)
    w_q1, w_k1, w_v1 = (np.asarray(w, np.float32) for w in (w_q1, w_k1, w_v1))
    w_q2, w_k2, w_v2 = (np.asarray(w, np.float32) for w in (w_q2, w_k2, w_v2))
    b_q1, b_q2 = np.asarray(b_q1, np.float32), np.asarray(b_q2, np.float32)
    b_v1, b_v2 = np.asarray(b_v1, np.float32), np.asarray(b_v2, np.float32)
    # folded scores weights: S1 = (A1 x1 + v1)^T x2 with A1 = Wk2^T Wq1.
    # lhsT packing is A^T = Wq^T Wk.
    a1t = w_q1.T @ w_k2
    a2t = w_q2.T @ w_k1
    # wpack order must match w_names: a1t, wv2t, a2t, wv1t
    wpack = np.ascontiguousarray(
        np.concatenate([a1t, w_v2.T, a2t, w_v1.T], axis=1).astype(np.float16)
    )
    # [128, ck, dir] packing (single DMA)
    bq = np.stack([w_k2.T @ b_q1, w_k1.T @ b_q2], axis=1)  # [C, 2]
    bq = np.ascontiguousarray(
        bq.reshape(CT, 128, 2).transpose(1, 0, 2).reshape(128, CT * 2)
    )

    in_maps = []
    for i in range(B):
        x1i = x1[i].reshape(C, N)
        x2i = x2[i].reshape(C, N)
        m = {
            "x1": np.ascontiguousarray(x1h[i].reshape(C, N)),
            "x2": np.ascontiguousarray(x2h[i].reshape(C, N)),
            # V bias folded into the residual: y = (x + bv) + sum_j w V0
            "x1t": np.ascontiguousarray(x1i.T + b_v2[None, :]),
            "x2t": np.ascontiguousarray(x2i.T + b_v1[None, :]),
            "wpack": wpack,
            "bq": bq,
        }
        in_maps.append(m)

    res = run_bass_kernel_spmd(nc, in_maps, list(range(B)), trace=_trace)
    if _trace:
        _CACHE["last_result"] = res

    y1 = np.empty((B, C, H, W), np.float32)
    y2 = np.empty((B, C, H, W), np.float32)
    for i in range(B):
        y1[i] = res.results[i]["y1t"].T.reshape(C, H, W)
        y2[i] = res.results[i]["y2t"].T.reshape(C, H, W)
    return y1, y2
